# revision 28
# baseline (speedup 1.0000x reference)
"""Trainium2 Bass kernel for nn_BCAblock_Anchor (bilateral window cross-attention block).

Sharding: spatial over image rows. 8 cores x 24 rows each (both batches on
every core); k/v inputs are passed with a +-4 row halo (zero padded at image
borders, matching the reference's zero padding of k/v). No collectives.

Per-core: 4 sequential passes of 12 image rows (2 batches x 2 sub-tiles).
Channel-on-partition [128c, pixels] slabs in a 200-wide x-padded flat layout
(4 zero cols each side) so every (dy,dx) window shift is a free-dim AP offset.
"""

import sys

sys.path.insert(0, "/opt/trn_rl_repo")

from contextlib import ExitStack

import numpy as np

import concourse.bass as bass
import concourse.bacc as bacc
import concourse.mybir as mybir
import concourse.tile as tile
from concourse.bass_utils import run_bass_kernel_spmd

F32 = mybir.dt.float32
BF16 = mybir.dt.bfloat16
I8 = mybir.dt.int8
F32R = mybir.dt.float32r
AF = mybir.ActivationFunctionType
OP = mybir.AluOpType

B, C, NH, WS = 2, 128, 4, 9
H, W, HC, MD = 192, 192, 32, 4
W2 = WS * WS                 # 81
NCORES = 8
RPC = H // NCORES            # 24 own rows per core
HR = RPC + 2 * MD            # 32 haloed rows per core
PW = W + 2 * MD              # 200 padded row width
NPIX = RPC * W               # 4608 own pixels per batch per core
NHPIX = HR * W               # 6144 haloed pixels per batch per core

SR = 12                      # rows per sub-tile pass
NST = RPC // SR              # 2 sub-tiles
SHR = SR + 2 * MD            # 20 haloed rows per pass
SNPIX = SR * W               # 2304
SNHPIX = SHR * W             # 3840
SSLAB = SHR * PW             # 4000
SNOWN = SR * PW              # 2400 own-window (incl x pads)
GUARD = 8
OWN0 = GUARD + MD * PW
CHSZ = 480
NCH = SNOWN // CHSZ          # 5

# packed layout of all [128, n] constant operands: one DRAM tensor, one DMA
# kv_w is duplicated with the per-tensor int8 dequant scale of x0/x1 folded in
CONST_SPEC = [
    ("eye128", 128), ("e128", 128), ("j128", 128), ("q_w", 128),
    ("kv_w0", 256), ("kv_w1", 256), ("proj_w0", 128), ("proj_w1", 128),
    ("fc1_w", 512),
    ("fc2_w0", 128), ("fc2_w1", 128), ("fc2_w2", 128), ("fc2_w3", 128),
    ("q_b2", 1), ("k_b2", 1), ("v_b2", 1), ("proj_b2", 1), ("fc1_b2", 4),
    ("fc2_b2", 1), ("n1w", 1), ("n1b", 1), ("n2w", 1), ("n2b", 1),
    ("scale128", 1), ("bias_d", W2), ("eps24", 1), ("eps6", 1),
]
CONST_OFF = {}
_off = 0
for _n, _w in CONST_SPEC:
    CONST_OFF[_n] = _off
    _off += _w
NCONST = _off


def _trace(ctx, tc, io):
    nc = tc.nc

    consts = ctx.enter_context(tc.tile_pool(name="consts", bufs=1))
    slabs = ctx.enter_context(tc.tile_pool(name="slabs", bufs=1))
    work = ctx.enter_context(tc.tile_pool(name="work", bufs=2))
    post = ctx.enter_context(tc.tile_pool(name="post", bufs=1))
    dloop = ctx.enter_context(tc.tile_pool(name="dloop", bufs=3))
    psum = ctx.enter_context(tc.tile_pool(name="psum", bufs=4, space="PSUM"))
    psumt = ctx.enter_context(tc.tile_pool(name="psumt", bufs=2, space="PSUM"))

    cpack = consts.tile([128, NCONST], F32, tag="cpack")
    nc.sync.dma_start(cpack[:], io["cpack"][:])

    def cslice(name):
        lo = CONST_OFF[name]
        return cpack[:, lo:lo + dict(CONST_SPEC)[name]]

    eye = cslice("eye128")
    e128f = cslice("e128")                       # block-diag ones, f32
    j128 = cslice("j128")                        # all 1/128 (LN mean)
    qw = cslice("q_w")
    pjw0 = cslice("proj_w0")
    pjw1 = cslice("proj_w1")
    f1w = cslice("fc1_w")
    f2ws = [cslice(f"fc2_w{g}") for g in range(4)]
    qb = cslice("q_b2")
    kb = cslice("k_b2")
    vb = cslice("v_b2")
    pjb = cslice("proj_b2")
    f1b = cslice("fc1_b2")
    f2b = cslice("fc2_b2")
    n1w = cslice("n1w")
    n1b = cslice("n1b")
    n2w = cslice("n2w")
    n2b = cslice("n2b")
    sc128 = cslice("scale128")
    bias_d = cslice("bias_d")
    eps24 = cslice("eps24")
    eps6 = cslice("eps6")

    # bf16 copies of transpose-identity and kv weights, converted on device
    eyeb_t = consts.tile([128, 128], BF16, tag="eye128b")
    nc.gpsimd.tensor_copy(eyeb_t[:], eye)
    eyeb = eyeb_t
    kvw0_t = consts.tile([128, 256], BF16, tag="kvw0")
    nc.gpsimd.tensor_copy(kvw0_t[:], cslice("kv_w0"))
    kvw1_t = consts.tile([128, 256], BF16, tag="kvw1")
    nc.gpsimd.tensor_copy(kvw1_t[:], cslice("kv_w1"))

    def l2norm_slab(t, n):
        """Per-head l2 normalize columns of a [128, n] channel-major tile."""
        csz = 512
        nchunks = (n + csz - 1) // csz
        for i in range(nchunks):
            lo = i * csz
            m = min(csz, n - lo)
            s = slice(lo, lo + m)
            sq = work.tile([128, csz], F32, tag="sq")
            nc.vector.tensor_mul(sq[:, :m], t[:, s], t[:, s])
            ps = psum.tile([128, csz], F32, tag="mm")
            nc.tensor.matmul(ps[:, :m], e128f[:], sq[:, :m])
            sd = work.tile([128, csz], F32, tag="sd")
            nc.scalar.activation(sd[:, :m], ps[:, :m], AF.Sqrt, bias=eps24[:])
            rn = work.tile([128, csz], F32, tag="rn")
            nc.vector.reciprocal(rn[:, :m], sd[:, :m])
            nc.vector.tensor_mul(t[:, s], t[:, s], rn[:, :m])

    def project(src_t, npix, w_ap, bias_t, out_tile):
        """out = (w.T @ src) + b, channel-major; w_ap [128, M<=128] bf16."""
        nchunks = (npix + 511) // 512
        for i in range(nchunks):
            lo = i * 512
            m = min(512, npix - lo)
            s = slice(lo, lo + m)
            ps = psum.tile([128, 512], F32, tag="mm")
            nc.tensor.matmul(ps[:, :m], w_ap, src_t[:, s])
            nc.vector.tensor_scalar_add(out_tile[:, s], ps[:, :m], bias_t[:])

    def restride(flat_t, slab_t, nrows, row0):
        """[128, nrows*192] -> padded slab rows row0.. via SBUF DMA."""
        src = flat_t[:, :nrows * W].rearrange("p (r w) -> p r w", r=nrows)
        dst = slab_t[:, GUARD:GUARD + SSLAB].rearrange(
            "p (r w) -> p r w", r=SHR)[:, row0:row0 + nrows, MD:MD + W]
        nc.sync.dma_start(dst, src)

    out_dram = io["out"]

    for b in range(B):
        for st in range(NST):
            # global input offsets for this pass
            hoff = (b * HR + st * SR) * W          # into x0h/x1h (haloed rows)
            toff = (b * RPC + st * SR) * W         # into xt / out rows

            # ---- slabs ----
            q_s = slabs.tile([128, SNOWN + 2 * GUARD], F32, tag="q_s")
            k0_s = slabs.tile([128, SSLAB + 2 * GUARD], F32, tag="k0_s")
            k1_s = slabs.tile([128, SSLAB + 2 * GUARD], F32, tag="k1_s")
            v0_s = slabs.tile([128, SSLAB + 2 * GUARD], BF16, tag="v0_s")
            v1_s = slabs.tile([128, SSLAB + 2 * GUARD], BF16, tag="v1_s")
            if b == 0 and st == 0:
                # pads/guards stay zero across passes: restrides only write
                # data columns and l2norm maps 0 -> 0 in place
                for t in (q_s, k0_s, k1_s, v0_s, v1_s):
                    nc.gpsimd.memset(t[:], 0.0)

            # ---- x0/x1 -> k/v slabs (int8 inputs, scale folded into kv_w) ----
            for (xin, kvw, k_t, v_t) in ((io["x0h"], kvw0_t, k0_s, v0_s),
                                         (io["x1h"], kvw1_t, k1_s, v1_s)):
                xu = slabs.tile([128, SNHPIX], BF16, tag="xu")
                for i in range(SNHPIX // 128):
                    x8 = post.tile([128, 128], I8, tag="tin8")
                    nc.sync.dma_start(
                        x8[:], xin[hoff + i * 128:hoff + (i + 1) * 128, :])
                    xt_ = post.tile([128, 128], BF16, tag="tin")
                    nc.gpsimd.tensor_copy(xt_[:], x8[:])
                    pt = psumt.tile([128, 128], BF16, tag="ptrb")
                    nc.tensor.matmul(pt[:], xt_[:], eyeb[:], is_transpose=True)
                    if i % 2 == 0:
                        nc.vector.tensor_copy(xu[:, i * 128:(i + 1) * 128], pt[:])
                    else:
                        nc.scalar.copy(xu[:, i * 128:(i + 1) * 128], pt[:])
                ku = slabs.tile([128, SNHPIX], F32, tag="ku")
                project(xu, SNHPIX, kvw[:, 0:128], kb, ku)
                vu = slabs.tile([128, SNHPIX], BF16, tag="vu")
                project(xu, SNHPIX, kvw[:, 128:256], vb, vu)
                restride(ku, k_t, SHR, 0)
                restride(vu, v_t, SHR, 0)
                l2norm_slab(k_t[:, GUARD:GUARD + SSLAB], SSLAB)

            # ---- xt -> q slab (+ keep f32 transposed copy for residual) ----
            xtu = slabs.tile([128, SNPIX], F32, tag="xtu")
            for i in range(SNPIX // 128):
                xt_ = post.tile([128, 128], BF16, tag="tin")
                nc.sync.dma_start(
                    xt_[:], io["xt"][toff + i * 128:toff + (i + 1) * 128, :])
                pt = psumt.tile([128, 128], BF16, tag="ptrb")
                nc.tensor.matmul(pt[:], xt_[:], eyeb[:], is_transpose=True)
                if i % 2 == 0:
                    nc.vector.tensor_copy(xtu[:, i * 128:(i + 1) * 128], pt[:])
                else:
                    nc.scalar.copy(xtu[:, i * 128:(i + 1) * 128], pt[:])
            qu = slabs.tile([128, SNPIX], F32, tag="vu")
            project(xtu, SNPIX, qw[:], qb, qu)
            # q slab: own rows only, [128, 12*200] + guards
            src = qu[:].rearrange("p (r w) -> p r w", r=SR)
            dstq = q_s[:, GUARD:GUARD + SNOWN].rearrange(
                "p (r w) -> p r w", r=SR)[:, :, MD:MD + W]
            nc.sync.dma_start(dstq, src)
            l2norm_slab(q_s[:, GUARD:GUARD + SNOWN], SNOWN)

            # ---- attention: 81 shifted passes over 5 chunks ----
            xb_s = slabs.tile([128, SNOWN], F32, tag="xu")
            xf_s = slabs.tile([128, SNOWN], F32, tag="ku")
            for ci in range(NCH):
                oo = ci * CHSZ
                o = OWN0 + oo                 # in k/v slab padded flat coords
                oq = GUARD + oo               # in q slab coords
                qc = q_s[:, oq:oq + CHSZ]
                xbc = xb_s[:, oo:oo + CHSZ]
                xfc = xf_s[:, oo:oo + CHSZ]
                zc = work.tile([128, CHSZ], F32, tag="zc")
                first = True
                for dy in range(-MD, MD + 1):
                    for dx in range(-MD, MD + 1):
                        d = (dy + MD) * WS + (dx + MD)
                        sh_b = o - dy * PW - dx   # k0/v0 at p-d
                        sh_f = o + dy * PW + dx   # k1/v1 at p+d
                        pr0 = dloop.tile([128, CHSZ], F32, tag="pr0")
                        nc.vector.tensor_mul(pr0[:], qc, k0_s[:, sh_b:sh_b + CHSZ])
                        pr1 = dloop.tile([128, CHSZ], F32, tag="pr1")
                        nc.vector.tensor_mul(pr1[:], qc, k1_s[:, sh_f:sh_f + CHSZ])
                        pl = psum.tile([128, CHSZ], F32, tag="mm")
                        nc.tensor.matmul(pl[:], e128f[:], pr0[:], start=True, stop=False)
                        nc.tensor.matmul(pl[:], e128f[:], pr1[:], start=False, stop=True)
                        # a = exp(scale*logit + bias_d); no max-subtraction
                        # needed: |scale*logit| <= 200, safe in fp32.
                        ar = dloop.tile([128, CHSZ], BF16, tag="ar")
                        nc.scalar.activation(ar[:], pl[:], AF.Exp,
                                             bias=bias_d[:, d:d + 1], scale=sc128[:])
                        t0 = dloop.tile([128, CHSZ], BF16, tag="t0")
                        nc.vector.tensor_mul(t0[:], ar[:], v0_s[:, sh_b:sh_b + CHSZ])
                        t1 = dloop.tile([128, CHSZ], BF16, tag="t1")
                        nc.gpsimd.tensor_mul(t1[:], ar[:], v1_s[:, sh_f:sh_f + CHSZ])
                        if first:
                            nc.vector.tensor_copy(zc[:], ar[:])
                            nc.vector.tensor_copy(xbc, t0[:])
                            nc.gpsimd.tensor_copy(xfc, t1[:])
                            first = False
                        else:
                            nc.vector.tensor_add(zc[:], zc[:], ar[:])
                            nc.vector.tensor_add(xbc, xbc, t0[:])
                            nc.gpsimd.tensor_add(xfc, xfc, t1[:])
                rz = work.tile([128, CHSZ], F32, tag="rz")
                nc.vector.reciprocal(rz[:], zc[:])
                nc.vector.tensor_mul(xbc, xbc, rz[:])
                nc.vector.tensor_mul(xfc, xfc, rz[:])

            # repack padded own-window -> unpadded [128, 2304]
            xbu = slabs.tile([128, SNPIX], F32, tag="xbu")
            xfu = slabs.tile([128, SNPIX], F32, tag="xfu")
            for (srct, dstt) in ((xb_s, xbu), (xf_s, xfu)):
                sv = srct[:].rearrange("p (r w) -> p r w", r=SR)[:, :, MD:MD + W]
                dv = dstt[:].rearrange("p (r w) -> p r w", r=SR)
                nc.sync.dma_start(dv, sv)

            # ---- proj + LN1 + residual; MLP + LN2 + residual ----
            def layernorm(y_t, w_t, b_t, out_t, m):
                pm = psum.tile([128, 512], F32, tag="mm")
                nc.tensor.matmul(pm[:, :m], j128[:], y_t[:, :m])
                xc = post.tile([128, 512], F32, tag="xc")
                nc.vector.tensor_sub(xc[:, :m], y_t[:, :m], pm[:, :m])
                sq = post.tile([128, 512], F32, tag="lsq")
                nc.vector.tensor_mul(sq[:, :m], xc[:, :m], xc[:, :m])
                pv = psum.tile([128, 512], F32, tag="mm")
                nc.tensor.matmul(pv[:, :m], j128[:], sq[:, :m])
                sd = post.tile([128, 512], F32, tag="lsd")
                nc.scalar.activation(sd[:, :m], pv[:, :m], AF.Sqrt, bias=eps6[:])
                rs = post.tile([128, 512], F32, tag="lrs")
                nc.vector.reciprocal(rs[:, :m], sd[:, :m])
                nc.vector.tensor_mul(xc[:, :m], xc[:, :m], rs[:, :m])
                nc.vector.tensor_scalar(out_t[:, :m], xc[:, :m], w_t[:], b_t[:],
                                        op0=OP.mult, op1=OP.add)

            xa = slabs.tile([128, SNPIX], F32, tag="xa")
            nchp = (SNPIX + 511) // 512
            for ci in range(nchp):
                lo = ci * 512
                m = min(512, SNPIX - lo)
                s = slice(lo, lo + m)
                pp = psum.tile([128, 512], F32, tag="mm")
                nc.tensor.matmul(pp[:, :m], pjw0[:], xbu[:, s], start=True, stop=False)
                nc.tensor.matmul(pp[:, :m], pjw1[:], xfu[:, s], start=False, stop=True)
                y = post.tile([128, 512], F32, tag="y")
                nc.vector.tensor_scalar_add(y[:, :m], pp[:, :m], pjb[:])
                ln = post.tile([128, 512], F32, tag="ln")
                layernorm(y, n1w, n1b, ln, m)
                nc.vector.tensor_add(xa[:, s], xtu[:, s], ln[:, :m])

                hts = []
                for g in range(4):
                    ph = psum.tile([128, 512], F32, tag="mm")
                    nc.tensor.matmul(ph[:, :m], f1w[:, g * 128:(g + 1) * 128], xa[:, s])
                    ht = post.tile([128, 512], F32, tag=f"ht{g}")
                    nc.scalar.activation(ht[:, :m], ph[:, :m], AF.Gelu,
                                         bias=f1b[:, g:g + 1])
                    hts.append(ht)
                po = psum.tile([128, 512], F32, tag="mm")
                for g in range(4):
                    nc.tensor.matmul(po[:, :m], f2ws[g][:], hts[g][:, :m],
                                     start=(g == 0), stop=(g == 3))
                y2 = post.tile([128, 512], F32, tag="y2")
                nc.vector.tensor_scalar_add(y2[:, :m], po[:, :m], f2b[:])
                ln2 = post.tile([128, 512], F32, tag="ln2")
                layernorm(y2, n2w, n2b, ln2, m)
                ot = post.tile([128, 512], F32, tag="oc")
                nc.vector.tensor_add(ot[:, :m], xa[:, s], ln2[:, :m])

                # transpose back and store this chunk (m is a multiple of 128)
                for i in range(m // 128):
                    pt = psumt.tile([128, 128], F32, tag="ptr")
                    nc.tensor.matmul(pt[:], ot[:, i * 128:(i + 1) * 128], eye[:],
                                     is_transpose=True)
                    og = work.tile([128, 128], BF16, tag="otb")
                    if i % 2 == 0:
                        nc.vector.tensor_copy(og[:], pt[:])
                    else:
                        nc.scalar.copy(og[:], pt[:])
                    row = toff + lo + i * 128
                    nc.sync.dma_start(out_dram[row:row + 128, :], og[:])


_CACHE = {}


def _get_program():
    if "prog" in _CACHE:
        return _CACHE["prog"]
    nc = bacc.Bacc("TRN2", target_bir_lowering=False, debug=False,
                   num_devices=NCORES)
    io = {}

    def din(name, shape, dtype=F32):
        io[name] = nc.dram_tensor(name, shape, dtype, kind="ExternalInput").ap()

    din("xt", [B * NPIX, C], BF16)
    din("x0h", [B * NHPIX, C], I8)
    din("x1h", [B * NHPIX, C], I8)
    din("cpack", [128, NCONST])
    io["out"] = nc.dram_tensor("out", [B * NPIX, C], BF16,
                               kind="ExternalOutput").ap()
    ctx = ExitStack()
    with ctx:
        tc = ctx.enter_context(tile.TileContext(nc, trace_sim=False))
        _trace(ctx, tc, io)
    nc.compile()
    _CACHE["prog"] = nc
    return nc


def _host_consts(q_b, kv_b, logit_scale, cpb_w1, cpb_b1, cpb_w2, proj_b,
                 norm1_w, norm1_b, fc1_b, fc2_b, norm2_w, norm2_b):
    """Precompute small constant operands (derived from weights only)."""
    gy, gx = np.meshgrid(np.arange(WS, dtype=np.float32) * 2.0,
                         np.arange(WS, dtype=np.float32) * 2.0, indexing="ij")
    t = np.stack([gy / (WS - 1) - 1.0, gx / (WS - 1) - 1.0], -1) * 8.0
    t = np.sign(t) * np.log2(np.abs(t) + 1.0) / np.log2(8.0)
    coords = t.reshape(-1, 2)
    hmid = np.maximum(coords @ cpb_w1 + cpb_b1, 0.0)
    bias = 16.0 / (1.0 + np.exp(-(hmid @ cpb_w2)))   # (81, NH)
    head_of_c = (np.arange(128) // HC)
    bias128 = np.ascontiguousarray(bias.T[head_of_c, :]).astype(np.float32)
    scale = np.exp(np.minimum(logit_scale.reshape(NH), np.log(100.0)))
    scale128 = scale[head_of_c].reshape(128, 1).astype(np.float32)

    e128 = np.zeros((128, 128), np.float32)
    for h in range(NH):
        e128[h * HC:(h + 1) * HC, h * HC:(h + 1) * HC] = 1.0
    return {
        "eye128": np.eye(128, dtype=np.float32),
        "e128": e128,
        "j128": np.full((128, 128), 1.0 / 128.0, np.float32),
        "q_b2": q_b.reshape(128, 1).astype(np.float32),
        "k_b2": kv_b[:128].reshape(128, 1).astype(np.float32),
        "v_b2": kv_b[128:].reshape(128, 1).astype(np.float32),
        "proj_b2": proj_b.reshape(128, 1).astype(np.float32),
        "fc1_b2": np.ascontiguousarray(fc1_b.reshape(4, 128).T).astype(np.float32),
        "fc2_b2": fc2_b.reshape(128, 1).astype(np.float32),
        "n1w": norm1_w.reshape(128, 1).astype(np.float32),
        "n1b": norm1_b.reshape(128, 1).astype(np.float32),
        "n2w": norm2_w.reshape(128, 1).astype(np.float32),
        "n2b": norm2_b.reshape(128, 1).astype(np.float32),
        "scale128": scale128,
        "bias_d": bias128,
        "eps24": np.full((128, 1), 1e-24, np.float32),
        "eps6": np.full((128, 1), 1e-6, np.float32),
    }


def _pack_consts(name2c):
    """Pack all [128, n] constant operands column-wise per CONST_SPEC."""
    cp = np.empty((128, NCONST), np.float32)
    for name, w in CONST_SPEC:
        lo = CONST_OFF[name]
        cp[:, lo:lo + w] = name2c[name]
    return cp


def _get_exec(name2arr):
    """Build (once) and cache the AOT-compiled sharded executable.

    Mirrors bass2jax.run_bass_via_pjrt but hoists jit construction,
    lowering and NEFF compile out of the per-call path, and donates
    device-generated zero output buffers instead of uploading them.
    """
    if "exec" in _CACHE:
        return _CACHE["exec"]

    import jax
    import jax.numpy as jnp
    from jax.sharding import Mesh, PartitionSpec, NamedSharding
    from jax.experimental.shard_map import shard_map
    from concourse.bass2jax import (_bass_exec_p, install_neuronx_cc_hook,
                                    partition_id_tensor)

    nc = _get_program()
    install_neuronx_cc_hook()
    partition_name = (nc.partition_id_tensor.name
                      if nc.partition_id_tensor else None)
    in_names, out_names, out_avals, zero_shapes = [], [], [], []
    for alloc in nc.m.functions[0].allocations:
        if not isinstance(alloc, mybir.MemoryLocationSet):
            continue
        aname = alloc.memorylocations[0].name
        if alloc.kind == "ExternalInput":
            if aname != partition_name:
                in_names.append(aname)
        elif alloc.kind == "ExternalOutput":
            shape = tuple(alloc.tensor_shape)
            dtype = mybir.dt.np(alloc.dtype)
            out_avals.append(jax.core.ShapedArray(shape, dtype))
            out_names.append(aname)
            zero_shapes.append((shape, dtype))
    n_params = len(in_names)
    n_outs = len(out_avals)
    all_names = list(in_names) + list(out_names)
    if partition_name is not None:
        all_names.append(partition_name)
    donate = tuple(range(n_params, n_params + n_outs))

    def _body(*args):
        operands = list(args)
        if partition_name is not None:
            operands.append(partition_id_tensor())
        outs = _bass_exec_p.bind(
            *operands, out_avals=tuple(out_avals),
            in_names=tuple(all_names), out_names=tuple(out_names),
            lowering_input_output_aliases=(),
            sim_require_finite=True, sim_require_nnan=True, nc=nc)
        return tuple(outs)

    devices = jax.devices()[:NCORES]
    mesh = Mesh(np.asarray(devices), ("core",))
    cshard = NamedSharding(mesh, PartitionSpec("core"))
    in_specs = (PartitionSpec("core"),) * (n_params + n_outs)
    out_specs = (PartitionSpec("core"),) * n_outs
    sharded = jax.jit(
        shard_map(_body, mesh=mesh, in_specs=in_specs,
                  out_specs=out_specs, check_rep=False),
        donate_argnums=donate, keep_unused=True)

    def gshape(shape, dtype):
        return jax.ShapeDtypeStruct((NCORES * shape[0],) + tuple(shape[1:]),
                                    dtype, sharding=cshard)

    concat_in = [name2arr[n] for n in in_names]
    zero_structs = [gshape(s, d) for (s, d) in zero_shapes]
    compiled = sharded.lower(*concat_in, *zero_structs).compile()

    make_zeros = jax.jit(
        lambda: tuple(jnp.zeros((NCORES * s[0],) + tuple(s[1:]), d)
                      for (s, d) in zero_shapes),
        out_shardings=(cshard,) * n_outs)

    _CACHE["shard"] = cshard
    _CACHE["exec"] = (compiled, make_zeros, in_names)
    return _CACHE["exec"]


def kernel(x0, x1, xt, q_w, q_b, kv_w, kv_b, logit_scale, cpb_w1, cpb_b1,
           cpb_w2, proj_w, proj_b, norm1_w, norm1_b, fc1_w, fc1_b, fc2_w,
           fc2_b, norm2_w, norm2_b, h, w):
    import ml_dtypes
    bf16 = ml_dtypes.bfloat16
    x0 = np.asarray(x0, np.float32).reshape(B, H, W, C)
    x1 = np.asarray(x1, np.float32).reshape(B, H, W, C)
    xt = np.asarray(xt, np.float32).reshape(B, H, W, C)

    import jax

    warm = "exec" in _CACHE
    if warm:  # dispatch on-device zero-output creation before host prep
        zs = _CACHE["exec"][1]()

    consts = _host_consts(np.asarray(q_b), np.asarray(kv_b),
                          np.asarray(logit_scale), np.asarray(cpb_w1),
                          np.asarray(cpb_b1), np.asarray(cpb_w2),
                          np.asarray(proj_b), np.asarray(norm1_w),
                          np.asarray(norm1_b), np.asarray(fc1_b),
                          np.asarray(fc2_b), np.asarray(norm2_w),
                          np.asarray(norm2_b))
    proj_w = np.asarray(proj_w, np.float32)
    fc2_w = np.asarray(fc2_w, np.float32)
    kv_w = np.asarray(kv_w, np.float32)

    # int8 quantization scales for x0/x1 (clip at 4 rms, folded into kv_w)
    def qscale(x):
        rms = float(np.sqrt(np.mean(np.square(x.reshape(-1)[::97],
                                              dtype=np.float64))))
        return min(float(np.abs(x.reshape(-1)[::7]).max()) * 1.05,
                   4.0 * rms) / 127.0

    s0, s1 = qscale(x0), qscale(x1)
    consts.update({
        "q_w": np.asarray(q_w, np.float32),
        "kv_w0": kv_w * s0, "kv_w1": kv_w * s1,
        "proj_w0": proj_w[0:128], "proj_w1": proj_w[128:256],
        "fc1_w": np.asarray(fc1_w, np.float32),
        "fc2_w0": fc2_w[0:128], "fc2_w1": fc2_w[128:256],
        "fc2_w2": fc2_w[256:384], "fc2_w3": fc2_w[384:512],
    })
    cpack = _pack_consts(consts)
    cpack_cat = np.tile(cpack, (NCORES, 1))

    # build globally concatenated per-core inputs directly; x0/x1 are
    # quantized to int8 during the copy, halo rows clipped at image
    # borders and zero-filled; device_put each as soon as it is ready
    # so H2D overlaps the remaining host prep
    def halo_cat_q(x, s):
        t = x * (1.0 / s)
        np.rint(t, out=t)
        np.clip(t, -127, 127, out=t)
        q = t.astype(np.int8)
        cat = np.empty((NCORES, B, HR, W, C), np.int8)
        for ci in range(NCORES):
            r0 = ci * RPC
            lo, hi = r0 - MD, r0 + RPC + MD
            clo, chi = max(lo, 0), min(hi, H)
            cat[ci, :, clo - lo:HR - (hi - chi)] = q[:, clo:chi]
            if lo < clo:
                cat[ci, :, :clo - lo] = 0
            if hi > chi:
                cat[ci, :, HR - (hi - chi):] = 0
        return cat.reshape(NCORES * B * NHPIX, C)

    shard = _CACHE["shard"] if warm else None

    def put(a):
        return jax.device_put(a, shard) if shard is not None else a

    d_x0 = put(halo_cat_q(x0, s0))
    d_x1 = put(halo_cat_q(x1, s1))
    xt_cat = np.empty((NCORES, B, RPC, W, C), bf16)
    for ci in range(NCORES):
        xt_cat[ci] = xt[:, ci * RPC:(ci + 1) * RPC]
    d_xt = put(xt_cat.reshape(NCORES * B * NPIX, C))
    d_cp = put(cpack_cat)

    name2arr = {"xt": d_xt, "x0h": d_x0, "x1h": d_x1, "cpack": d_cp}

    compiled, make_zeros, in_names = _get_exec(name2arr)
    if not warm:
        zs = make_zeros()
    out_arrs = compiled(*[name2arr[n] for n in in_names], *zs)
    outg = np.asarray(out_arrs[0])  # (NCORES*B*NPIX, C) bf16

    out = np.empty((B, H, W, C), np.float32)
    og = outg.reshape(NCORES, B, RPC, W, C)
    for ci in range(NCORES):
        out[:, ci * RPC:(ci + 1) * RPC] = og[ci]
    return out.reshape(B, H * W, C)

